# revision 1
# baseline (speedup 1.0000x reference)
"""Trainium2 Bass kernel for sparse (top-k) multi-head causal attention.

Problem (hardcoded shapes, from the reference):
  B=32, S=512, D=512, H=8, DK=64, k_index=5 (any k<=8 supported)
  out = TopKCausalAttention(q, k, v; w_q..w_o, b_q..b_o)

Sharding: data-parallel over batch across 8 NeuronCores (4 batches/core).

Per-core algorithm (all on one core, per batch b and head h):
  qhT[d, r] = (w_q/8)^T-projection of q (transposed layout, d on partitions)
  khT[d, c] likewise; vh[r, d] in natural layout.
  scores_psum[r-tile, :] = qhT.T @ khT  (+ bf16 identity-matmul adds the
      strictly-causal -1e32 mask on the diagonal tile; upper tiles skipped)
  e = exp(scores)                 (ACT, PSUM->SBUF)
  top8 = vector.max(e)            (top-8 per row, one DVE op)
  tau = top8[:, k-1]; rows < k get tau := 0 (keep everything valid)
  Z = sum(top8[:, :k]) per row    (rows < k: full-row sum; row 0: Z := 1)
  p = (e >= tau) * e * (1/Z)      (DVE scalar_tensor_tensor + GPSIMD
                                   tensor_scalar; exact top-k by value
                                   threshold, matching the reference
                                   `probs >= thresh` semantics)
  pT via PE transposes banked 4-wide into one PSUM tile, one wide
  evacuation per column-tile, then one wide attnT matmul per ci:
  attnT[d, r>=ci*128] += vh_ci_headslice.T @ pT_ci   (fp16)
  y[r, :] = sum_hp attnT_hp-slice.T @ w_o-slice (+ b_o)  -> DRAM out

  dtypes: q/k projections + QK^T in fp32 (top-k selection is
  discontinuous — lower precision flips selected indices; fp32r and
  fp16x2-split were measured to flip rows on the graded data). The v/p
  path runs in fp16 (same 1 cyc/row PE rate as bf16, 8x less rounding
  error; absmax ~5e-4 of scale).
"""

import math
import os

os.environ.setdefault("MYCRO_LOCAL_CACHE", "1")

from contextlib import ExitStack

import numpy as np

import concourse.bass as bass
import concourse.bacc as bacc
import concourse.mybir as mybir
import concourse.tile as tile
from concourse.bass_utils import run_bass_kernel_spmd

B, S, D, H = 32, 512, 512, 8
DK = D // H  # 64
NCORES = 8
BC = B // NCORES  # batches per core
RT = S // 128  # row tiles per sequence
FT = D // 128  # feature tiles
NEG = -1.0e32

F32 = mybir.dt.float32
BF16 = mybir.dt.bfloat16
F16 = mybir.dt.float16

_last_nc = None

# dtype config knobs (tweakable for perf iteration)
CFG = {
    "qk_dt": F32,    # q/k projection + QK^T matmuls (selection-critical: f32)
    "v_dt": F16,     # v projection / attnT / y matmuls (smooth path)
    "p_dt": F16,     # dtype of normalized probs (transpose + pV path)
    "trace": False,
}


def _build_program(k_index: int, has_bias: dict):
    """Builds the per-core Bass program. Returns (nc, input_names)."""
    nc = bacc.Bacc(
        "TRN2", target_bir_lowering=False, debug=False, num_devices=NCORES
    )

    QKDT = CFG["qk_dt"]
    VDT = CFG["v_dt"]
    PDT = CFG["p_dt"]

    # --- DRAM I/O -------------------------------------------------------
    qT = nc.dram_tensor("qT", (BC, D, S), QKDT, kind="ExternalInput").ap()
    kT = nc.dram_tensor("kT", (BC, D, S), QKDT, kind="ExternalInput").ap()
    vT = nc.dram_tensor("vT", (BC, D, S), VDT, kind="ExternalInput").ap()
    wq = nc.dram_tensor("wq", (D, D), QKDT, kind="ExternalInput").ap()
    wk = nc.dram_tensor("wk", (D, D), QKDT, kind="ExternalInput").ap()
    wv = nc.dram_tensor("wv", (D, D), VDT, kind="ExternalInput").ap()
    wo = nc.dram_tensor("wo", (D, D), VDT, kind="ExternalInput").ap()
    bias_aps = {}
    for name in ("bq", "bk", "bv", "bo"):
        if has_bias[name]:
            bias_aps[name] = nc.dram_tensor(
                name, (1, D), F32, kind="ExternalInput"
            ).ap()
    out = nc.dram_tensor("out", (BC, S, D), F32, kind="ExternalOutput").ap()

    # --- inline constants ----------------------------------------------
    ident_np = np.eye(128, dtype=np.float32)
    # additive strict-causal mask for a diagonal tile: M[r, c] = NEG if c >= r
    mask_np = np.where(
        np.arange(128)[None, :] >= np.arange(128)[:, None], NEG, 0.0
    ).astype(np.float32)
    ident_p = nc.inline_tensor(
        ident_np.astype(mybir.dt.np(PDT)), name="identp"
    ).ap()
    ident_b = nc.inline_tensor(
        ident_np.astype(mybir.dt.np(BF16)), name="identb"
    ).ap()
    maskT_b = nc.inline_tensor(
        mask_np.T.copy().astype(mybir.dt.np(BF16)), name="maskT"
    ).ap()
    ones_row = nc.inline_tensor(
        np.ones((1, S), dtype=np.float32), name="onesrow"
    ).ap()

    with tile.TileContext(nc) as tc, ExitStack() as ctx:
        # ---------------- pools ----------------
        consts = ctx.enter_context(tc.tile_pool(name="consts", bufs=1))
        xpool = ctx.enter_context(tc.tile_pool(name="xpool", bufs=2))
        projpool = ctx.enter_context(tc.tile_pool(name="projpool", bufs=2))
        epool = ctx.enter_context(tc.tile_pool(name="epool", bufs=20))
        ppool = ctx.enter_context(tc.tile_pool(name="ppool", bufs=8))
        pnpool = ctx.enter_context(tc.tile_pool(name="pnpool", bufs=12))
        ptpool = ctx.enter_context(tc.tile_pool(name="ptpool", bufs=12))
        smallpool = ctx.enter_context(tc.tile_pool(name="smallpool", bufs=4))
        atpool = ctx.enter_context(tc.tile_pool(name="atpool", bufs=3))
        ypool = ctx.enter_context(tc.tile_pool(name="ypool", bufs=3))

        ps_proj = ctx.enter_context(tc.tile_pool(name="ps_proj", bufs=2, space="PSUM"))
        ps_sc = ctx.enter_context(tc.tile_pool(name="ps_sc", bufs=2, space="PSUM"))
        ps_pt = ctx.enter_context(tc.tile_pool(name="ps_pt", bufs=1, space="PSUM"))
        ps_at = ctx.enter_context(tc.tile_pool(name="ps_at", bufs=2, space="PSUM"))
        ps_y = ctx.enter_context(tc.tile_pool(name="ps_y", bufs=1, space="PSUM"))

        # ---------------- resident constants ----------------
        # q/k weights first, then batch 0's activations, then the rest of
        # the weights: on the DMA queue this lets the first projection
        # matmuls start ~8us earlier instead of waiting for all 16 weight
        # tiles to land.
        wq_sb = [consts.tile_from(wq[ft * 128:(ft + 1) * 128, :], name=f"wq{ft}")
                 for ft in range(FT)]
        _xq0 = [xpool.tile_from(qT[0, ft * 128:(ft + 1) * 128, :],
                                name=f"xq{ft}") for ft in range(FT)]
        wk_sb = [consts.tile_from(wk[ft * 128:(ft + 1) * 128, :], name=f"wk{ft}")
                 for ft in range(FT)]
        preloaded = {}
        preloaded[0] = (
            _xq0,
            [xpool.tile_from(kT[0, ft * 128:(ft + 1) * 128, :],
                             name=f"xk{ft}") for ft in range(FT)],
            [xpool.tile_from(vT[0, ft * 128:(ft + 1) * 128, :],
                             name=f"xv{ft}") for ft in range(FT)],
        )
        wv_sb = [consts.tile_from(wv[ft * 128:(ft + 1) * 128, :], name=f"wv{ft}")
                 for ft in range(FT)]
        wo_sb = [consts.tile_from(wo[dt * 128:(dt + 1) * 128, :], name=f"wo{dt}")
                 for dt in range(FT)]
        identp_sb = consts.tile_from(ident_p, name="identp_sb")
        if PDT == mybir.dt.float32r:
            identp_sb = identp_sb.bitcast(PDT)  # same 4-byte bits as f32
        elif PDT == BF16:
            identp_sb = None  # use identb_sb at the call site
        identb_sb = consts.tile_from(ident_b, name="identb_sb")
        maskT_sb = consts.tile_from(maskT_b, name="maskT_sb")
        ones_sb = consts.tile_from(ones_row, name="ones_sb")
        bias_sb = {
            nm: consts.tile_from(ap, name=f"{nm}_sb") for nm, ap in bias_aps.items()
        }

        Exp = mybir.ActivationFunctionType.Exp
        AO = mybir.AluOpType

        def emit_proj(b, defer_v=False):
            """Loads + q/k/v projections for batch b."""
            if b in preloaded:
                xq, xk, xv = preloaded.pop(b)
            else:
                xq = [xpool.tile_from(qT[b, ft * 128:(ft + 1) * 128, :],
                                      name=f"xq{ft}") for ft in range(FT)]
                xk = [xpool.tile_from(kT[b, ft * 128:(ft + 1) * 128, :],
                                      name=f"xk{ft}") for ft in range(FT)]
                xv = [xpool.tile_from(vT[b, ft * 128:(ft + 1) * 128, :],
                                      name=f"xv{ft}") for ft in range(FT)]
            qhT, khT, vh = [], [], []
            # interleaved per dt so the first head-pair (dt=0) has both its
            # qhT and khT tiles after two projection groups, not five
            for dt in range(FT):
                for which, w_sb, xs, bkey, outl in (
                        ("q", wq_sb, xq, "bq", qhT), ("k", wk_sb, xk, "bk", khT)):
                    ps = ps_proj.tile([128, S], F32, name="psq", tag="psproj")
                    nbias = bkey in bias_sb
                    for ft in range(FT):
                        nc.tensor.matmul(
                            ps, w_sb[ft][:, dt * 128:(dt + 1) * 128], xs[ft],
                            start=(ft == 0), stop=(ft == FT - 1 and not nbias))
                    if nbias:
                        nc.tensor.matmul(
                            ps, bias_sb[bkey][0:1, dt * 128:(dt + 1) * 128],
                            ones_sb, start=False, stop=True)
                    t = projpool.tile([128, S], QKDT, name=f"{which}hT{dt}",
                                      tag=f"{which}hT{dt}")
                    nc.scalar.copy(t, ps)
                    outl.append(t)
            def do_vproj(rts=range(RT)):
                for rt in rts:
                    ps = ps_proj.tile([128, D], F32, name="psv", tag="psproj")
                    nbias = "bv" in bias_sb
                    for ft in range(FT):
                        nc.tensor.matmul(
                            ps, xv[ft][:, rt * 128:(rt + 1) * 128], wv_sb[ft],
                            start=(ft == 0), stop=(ft == FT - 1 and not nbias))
                    if nbias:
                        nc.tensor.matmul(
                            ps, ones_sb[0:1, 0:128], bias_sb["bv"],
                            start=False, stop=True)
                    t = projpool.tile([128, D], VDT, name=f"vh{rt}", tag=f"vh{rt}")
                    nc.scalar.copy(t, ps)
                    vh.append(t)
                return vh
            if defer_v:
                return qhT, khT, do_vproj
            return qhT, khT, do_vproj()

        def emit_headpair(hp, qhT, khT, vh):
            """Scores / top-k softmax / transposes / attnT for one head pair.

            The two heads occupy partition halves 0:64 / 64:128 of qhT/khT, so
            their K=64 QK matmuls land in different PE row groups; issuing
            them back-to-back lets them run concurrently. The same applies to
            the M=64 attnT matmuls (different column groups), interleaved at
            the end.
            """
            etiles = [[None] * RT, [None] * RT]
            zfulls = [None, None]
            top8s = []
            for hh in range(2):
                top8s.append(smallpool.tile(
                    [128, RT * 8], F32, name=f"top8{hh}", tag=f"top8{hh}"))
            for ri in range(RT):
                w = (ri + 1) * 128
                spss = []
                # both heads' K=64 QK matmuls first (disjoint PE row groups ->
                # array-level concurrency), then the full-K mask matmuls which
                # would otherwise serialize them
                for hh in range(2):
                    po = hh * 64
                    sps = ps_sc.tile([128, S], F32, name="sps", tag="sps")
                    nc.tensor.matmul(
                        sps[:, 0:w],
                        qhT[hp][po:po + 64, ri * 128:(ri + 1) * 128],
                        khT[hp][po:po + 64, 0:w],
                        start=True, stop=False)
                    spss.append(sps)
                for hh in range(2):
                    nc.tensor.matmul(
                        spss[hh][:, ri * 128:(ri + 1) * 128],
                        maskT_sb, identb_sb, start=False, stop=True)
                for hh in range(2):
                    e = epool.tile([128, S], F32, name="e", tag="e")
                    if ri == 0:
                        zf = smallpool.tile(
                            [128, 1], F32, name=f"zfull{hh}", tag=f"zfull{hh}")
                        zfulls[hh] = zf
                        nc.scalar.activation(
                            e[:, 0:w], spss[hh][:, 0:w], Exp, accum_out=zf)
                    else:
                        nc.scalar.activation(e[:, 0:w], spss[hh][:, 0:w], Exp)
                    nc.vector.max(
                        out=top8s[hh][:, ri * 8:(ri + 1) * 8], in_=e[:, 0:w])
                    etiles[hh][ri] = e
            ptrows = [[None] * RT, [None] * RT]
            for hh in range(2):
                top8 = top8s[hh]
                # thresholds + normalizers (batched across row-tiles)
                zk = smallpool.tile([128, RT], F32, name="zk", tag="zk")
                nc.vector.reduce_sum(
                    zk, top8.rearrange("p (r e) -> p r e", e=8)[:, :, 0:k_index],
                    axis=mybir.AxisListType.X)
                nc.vector.tensor_copy(zk[0:k_index, 0:1], zfulls[hh][0:k_index, :])
                nc.vector.memset(zk[0:1, 0:1], 1.0)
                # rows < k keep every valid entry: tau := 0
                nc.vector.memset(top8[0:k_index, k_index - 1:k_index], 0.0)
                rz = smallpool.tile([128, RT], F32, name="rz", tag="rz")
                nc.vector.reciprocal(rz, zk)

                # masked, normalized probs
                pns = []
                for ri in range(RT):
                    w = (ri + 1) * 128
                    e = etiles[hh][ri]
                    tau = top8[:, ri * 8 + k_index - 1: ri * 8 + k_index]
                    pu = ppool.tile([128, S], F32, name="pu", tag="pu")
                    nc.vector.scalar_tensor_tensor(
                        pu[:, 0:w], e[:, 0:w], tau, e[:, 0:w],
                        op0=AO.is_ge, op1=AO.mult)
                    pn = pnpool.tile([128, S], PDT, name="pn", tag="pn")
                    nc.gpsimd.tensor_scalar(
                        pn[:, 0:w], pu[:, 0:w], rz[:, ri:ri + 1], None,
                        op0=AO.mult)
                    pns.append(pn)
                # transpose p per column-tile: bank 4 PE transposes into one
                # PSUM tile, then one wide evacuation per ci
                for ci in range(RT):
                    wv_ = (RT - ci) * 128
                    ptb = ps_pt.tile([128, S], PDT, name="ptb", tag="ptb")
                    for ri in range(ci, RT):
                        nc.tensor.transpose(
                            ptb[:, (ri - ci) * 128:(ri - ci + 1) * 128],
                            pns[ri][:, ci * 128:(ci + 1) * 128],
                            identb_sb if PDT == BF16 else identp_sb)
                    ptrow = ptpool.tile([128, S], PDT, name="ptrow", tag="ptrow")
                    if ci % 2 == 0:
                        nc.vector.tensor_copy(ptrow[:, 0:wv_], ptb[:, 0:wv_])
                    else:
                        nc.scalar.copy(ptrow[:, 0:wv_], ptb[:, 0:wv_])
                    ptrows[hh][ci] = ptrow
            # attnT: one wide matmul per (ci, head); the two heads' M=64
            # matmuls hit different column groups -> interleave for concurrency
            def finish(vh):
                at_ps = ps_at.tile([128, S], F32, name="atps", tag="atps")
                for ci in range(RT):
                    wv_ = (RT - ci) * 128
                    for hh in range(2):
                        h = 2 * hp + hh
                        po = hh * 64
                        nc.tensor.matmul(
                            at_ps[po:po + 64, ci * 128:S],
                            vh[ci][:, h * DK:(h + 1) * DK],
                            ptrows[hh][ci][:, 0:wv_],
                            start=(ci == 0), stop=(ci == RT - 1),
                            skip_group_check=True)
                at = atpool.tile([128, S], VDT, name=f"at{hp}", tag=f"at{hp}")
                nc.scalar.copy(at, at_ps)
                return at
            if vh is None:
                return finish
            return finish(vh)

        def emit_y(b, attnT_sb):
            for ri in range(RT):
                yps = ps_y.tile([128, D], F32, name="yps", tag="yps")
                nbias = "bo" in bias_sb
                for hp in range(FT):
                    nc.tensor.matmul(
                        yps, attnT_sb[hp][:, ri * 128:(ri + 1) * 128], wo_sb[hp],
                        start=(hp == 0), stop=(hp == FT - 1 and not nbias))
                if nbias:
                    nc.tensor.matmul(
                        yps, ones_sb[0:1, 0:128], bias_sb["bo"],
                        start=False, stop=True)
                y = ypool.tile([128, D], F32, name="y", tag="y")
                nc.scalar.copy(y, yps)
                nc.scalar.dma_start(out[b, ri * 128:(ri + 1) * 128, :], y)

        for b in range(BC):
            last = b == BC - 1
            qhT, khT, vh = emit_proj(b, defer_v=last)
            attnT_sb = []
            if last:
                # cooldown filler: last batch's v-projection groups spread
                # one per head-pair scores phase, filling PE gaps that no
                # next-batch projections exist to fill; the deferred attnT
                # finishes are dependency-driven and emitted afterwards
                do_v = vh
                fins = []
                vh = None
                for hp in range(FT):
                    fins.append(emit_headpair(hp, qhT, khT, None))
                    vh = do_v(rts=[hp])
                attnT_sb = [fin(vh) for fin in fins]
            else:
                for hp in range(FT):
                    attnT_sb.append(emit_headpair(hp, qhT, khT, vh))
            emit_y(b, attnT_sb)

    nc.compile()
    return nc


def kernel(**inputs):
    q = np.asarray(inputs["q"], np.float32)
    k = np.asarray(inputs["k"], np.float32)
    v = np.asarray(inputs["v"], np.float32)
    w_q = np.asarray(inputs["w_q"], np.float32)
    w_k = np.asarray(inputs["w_k"], np.float32)
    w_v = np.asarray(inputs["w_v"], np.float32)
    w_o = np.asarray(inputs["w_o"], np.float32)
    b_q = np.asarray(inputs["b_q"], np.float32)
    b_k = np.asarray(inputs["b_k"], np.float32)
    b_v = np.asarray(inputs["b_v"], np.float32)
    b_o = np.asarray(inputs["b_o"], np.float32)
    k_index = int(np.asarray(inputs["k_index"]))
    assert 1 <= k_index <= 8, f"kernel supports k_index<=8, got {k_index}"

    # fold the 1/sqrt(DK) score scaling into the q projection (exact: 2^-3)
    scale = np.float32(1.0 / math.sqrt(DK))
    w_qs = (w_q * scale).astype(np.float32)
    b_qs = (b_q * scale).astype(np.float32)

    has_bias = {
        "bq": bool(np.any(b_qs)),
        "bk": bool(np.any(b_k)),
        "bv": bool(np.any(b_v)),
        "bo": bool(np.any(b_o)),
    }

    nc = _build_program(k_index, has_bias)
    global _last_nc
    _last_nc = nc

    npq = mybir.dt.np(CFG["qk_dt"])
    npv = mybir.dt.np(CFG["v_dt"])
    shared = {
        "wq": np.ascontiguousarray(w_qs.astype(npq)),
        "wk": np.ascontiguousarray(w_k.astype(npq)),
        "wv": np.ascontiguousarray(w_v.astype(npv)),
        "wo": np.ascontiguousarray(w_o.astype(npv)),
    }
    for nm, arr in (("bq", b_qs), ("bk", b_k), ("bv", b_v), ("bo", b_o)):
        if has_bias[nm]:
            shared[nm] = np.ascontiguousarray(arr.reshape(1, D).astype(np.float32))

    in_maps = []
    for c in range(NCORES):
        sl = slice(c * BC, (c + 1) * BC)
        in_maps.append(dict(
            shared,
            qT=np.ascontiguousarray(q[sl].transpose(0, 2, 1).astype(npq)),
            kT=np.ascontiguousarray(k[sl].transpose(0, 2, 1).astype(npq)),
            vT=np.ascontiguousarray(v[sl].transpose(0, 2, 1).astype(npv)),
        ))

    res = run_bass_kernel_spmd(
        nc, in_maps, core_ids=list(range(NCORES)), trace=CFG["trace"]
    )
    out = np.concatenate([r["out"] for r in res.results], axis=0)
    kernel.last_result = res
    return out

